# revision 21
# baseline (speedup 1.0000x reference)
"""AGSRNet Trainium2 kernel.

Host (CPU, exact mirror of the reference for bit-identical top_k / eigh):
  - adjacency normalization, graph U-Net (-> net_outs, start_outs), eigh(A) -> U
Device (8 NeuronCores, one SPMD Bass launch, tensor-parallel over hr columns):
  - M = U.T @ net_outs            (column-sharded)
  - adjT rows = |M_c.T @ a.T|     (+ diag=1)       -> AllGather -> B = adj.T
  - Z rows   = (B[:,cs]).T @ B    (|.|, diag=1)    -> AllGather -> Zf
  - T1 = Zf.T @ gc1[:,cs]
  - h1T rows = relu(T1.T @ B)                      -> AllGather -> H1f
  - T2 rows  = (H1f[:,cs]).T @ gc2                 -> AllGather -> T2f
  - X = 0.5*relu((B[:,cs]).T @ T2f)   (h2 rows, halved)
  - z rows   = 0.5*relu((T2f[:,cs]).T @ B) + X     (diag fixed on host)
All device matmuls run in bf16 with fp32 accumulation.
"""

import numpy as np

LR = 1024
HR = 2048
HID = 1024
NCORES = 8
W = HR // NCORES          # 256 columns of the hr dimension per core
WH = HID // NCORES        # 128 columns of the hidden dim per core

KS = [0.9, 0.7, 0.6, 0.5]

_CACHE = {}

TRACE = False
LAST_EXEC_NS = None


# --------------------------------------------------------------------------
# Host prefix: exact eager-jax-on-CPU mirror of the reference up to net_outs,
# plus eigh(A).  Must follow the reference ops verbatim so that top_k index
# selection and eigenvector signs match the oracle bit-for-bit.
# --------------------------------------------------------------------------
def _host_prefix(lr, start_w, start_b, down_w, down_b, pool_w, pool_b,
                 bottom_w, bottom_b, up_w, up_b, end_w, end_b):
    import jax
    import jax.numpy as jnp

    cpu = jax.devices("cpu")[0]
    with jax.default_device(cpu):
        lr = jnp.asarray(lr)
        n = lr.shape[0]
        r = lr.sum(1) ** -0.5
        r = jnp.where(jnp.isinf(r), 0.0, r)
        A = (lr * r[None, :]).T * r[None, :]
        X = jnp.eye(n, dtype=lr.dtype)

        def _gcn(Ai, X, Wm, b):
            return (Ai @ X) @ Wm + b

        X = _gcn(A, X, jnp.asarray(start_w), jnp.asarray(start_b))
        start_outs = X
        org_X = X
        adj_ms, idxs, downs = [], [], []
        Ai = A
        for i in range(4):
            X = _gcn(Ai, X, jnp.asarray(down_w[i]), jnp.asarray(down_b[i]))
            adj_ms.append(Ai)
            downs.append(X)
            scores = jax.nn.sigmoid(
                (X @ jnp.asarray(pool_w[i]) + jnp.asarray(pool_b[i])) / 100.0)
            k = int(KS[i] * Ai.shape[0])
            vals, idx = jax.lax.top_k(scores, k)
            X = X[idx] * vals[:, None]
            Ai = Ai[idx][:, idx]
            idxs.append(idx)
        X = _gcn(Ai, X, jnp.asarray(bottom_w), jnp.asarray(bottom_b))
        for i in range(4):
            j = 3 - i
            Aj, idx = adj_ms[j], idxs[j]
            Xu = jnp.zeros((Aj.shape[0], X.shape[1]), X.dtype).at[idx].set(X)
            X = _gcn(Aj, Xu, jnp.asarray(up_w[i]), jnp.asarray(up_b[i])) + downs[j]
        X = jnp.concatenate([X, org_X], axis=1)
        net_outs = _gcn(A, X, jnp.asarray(end_w), jnp.asarray(end_b))

        _, U = jnp.linalg.eigh(A, UPLO='U', symmetrize_input=False)

        return (np.asarray(net_outs), np.asarray(start_outs), np.asarray(U),
                np.asarray(A))


# --------------------------------------------------------------------------
# Device graph
# --------------------------------------------------------------------------
def _build_nc():
    import concourse.bass as bass
    import concourse.mybir as mybir
    import concourse.tile as tile
    from concourse import bacc
    from concourse.bass import ts as bts
    from concourse.kernels.tile_matmul import (
        composable_matmul_tile_kernel, dma_from_dram_kxm, dma_from_dram_kxn,
        dma_to_dram_mxn, accumulate_dma_from_dram_mxn, ShapeInfo)

    f32 = mybir.dt.float32
    bf16 = mybir.dt.bfloat16
    i32 = mybir.dt.int32
    AF = mybir.ActivationFunctionType
    ALU = mybir.AluOpType

    nc = bacc.Bacc("TRN2", target_bir_lowering=False, debug=False,
                   num_devices=NCORES)

    # ---- external I/O (per-core) ----
    U_in = nc.dram_tensor("u", [LR, LR], bf16, kind="ExternalInput")
    NOc = nc.dram_tensor("netouts_c", [LR, W], bf16, kind="ExternalInput")
    AT = nc.dram_tensor("at", [LR, HR], bf16, kind="ExternalInput")
    GC1c = nc.dram_tensor("gc1c", [HR, WH], bf16, kind="ExternalInput")
    GC2 = nc.dram_tensor("gc2", [HID, HR], bf16, kind="ExternalInput")
    DM = nc.dram_tensor("dmask", [W, HR], bf16, kind="ExternalInput")

    ADJT_OUT = nc.dram_tensor("adjt", [W, HR], f32, kind="ExternalOutput")
    Z_OUT = nc.dram_tensor("zrows", [W, HR], f32, kind="ExternalOutput")

    # ---- internal DRAM ----
    # hr columns split into NCH chunks of CW so each AllGather pipelines
    # against the producing / consuming matmuls.
    NCH = 4
    CW = HR // NCH

    Mc = nc.dram_tensor("Mc", [LR, W], bf16)
    AJ = [nc.dram_tensor(f"AJ{k}", [W, CW], bf16) for k in range(NCH)]
    BfC = [nc.dram_tensor(f"Bf{k}", [HR, CW], bf16, addr_space="Shared")
           for k in range(NCH)]
    BAin = nc.dram_tensor("BAin", [HR, W], bf16)
    Bcs = nc.dram_tensor("Bcs", [HR, W], bf16)
    ZcC = [nc.dram_tensor(f"Zc{k}", [W, CW], bf16) for k in range(NCH)]
    ZfC = [nc.dram_tensor(f"Zf{k}", [HR, CW], bf16, addr_space="Shared")
           for k in range(NCH)]
    T1c = nc.dram_tensor("T1c", [HR, WH], bf16)
    H1cC = [nc.dram_tensor(f"H1c{k}", [WH, CW], bf16) for k in range(NCH)]
    HAin = nc.dram_tensor("HAin", [HID, W], bf16)
    H1cs = nc.dram_tensor("H1cs", [HID, W], bf16)
    T2cC = [nc.dram_tensor(f"T2c{k}", [W, CW], bf16) for k in range(NCH)]
    T2fC = [nc.dram_tensor(f"T2f{k}", [HR, CW], bf16, addr_space="Shared")
            for k in range(NCH)]
    TAin = nc.dram_tensor("TAin", [HR, W], bf16)
    T2cs = nc.dram_tensor("T2cs", [HR, W], bf16)
    Xc = nc.dram_tensor("Xc", [W, HR], f32)

    RG = [list(range(NCORES))]

    with tile.TileContext(nc) as tc:
        with (
            tc.tile_pool(name="const", bufs=1) as const,
            tc.tile_pool(name="aux", bufs=3) as aux,
            tc.tile_pool(name="kxm", bufs=6) as kxm_pool,
            tc.tile_pool(name="kxn", bufs=6) as kxn_pool,
        ):
            # zero bias for activations
            zbias = const.tile([128, 1], f32)
            nc.any.memset(zbias[:], 0.0)

            # masks resident in SBUF: D and OM = 1 - D, as [128, 2, HR]
            dm_sb = const.tile([128, W // 128, HR], bf16)
            nc.sync.dma_start(
                dm_sb[:], DM.ap().rearrange("(s p) n -> p s n", p=128))
            om_sb = const.tile([128, W // 128, HR], bf16)
            nc.vector.tensor_scalar(om_sb[:], dm_sb[:], -1.0, 1.0,
                                    ALU.mult, ALU.add)

            # PSUM -> SBUF evictions on the vector engine (DVE, ~4x faster
            # than ACT activation copies)
            def dve_copy(nc_, psum, sbuf, md):
                nc_.vector.tensor_copy(sbuf[:], psum[:])

            def dve_abs(nc_, psum, sbuf, md):
                # |x| = max(x, -x): negate into sbuf, then max with psum
                nc_.vector.tensor_scalar(sbuf[:], psum[:], -1.0, None,
                                         ALU.mult)
                nc_.vector.tensor_tensor(sbuf[:], sbuf[:], psum[:], ALU.max)

            def dve_relu(nc_, psum, sbuf, md):
                nc_.vector.tensor_scalar(sbuf[:], psum[:], 0.0, None, ALU.max)

            def dve_relu_half(nc_, psum, sbuf, md):
                nc_.vector.tensor_scalar(sbuf[:], psum[:], 0.0, 0.5,
                                         ALU.max, ALU.mult)

            def mmk(kxm_ap, kxn_ap, mxn_ap, reducer=dve_copy, post=None,
                    accum_ap=None, kxn_cache_sb=None, psum_bufs=2):
                kxm_producer, kxm_shape = dma_from_dram_kxm(kxm_pool, kxm_ap)
                if kxn_cache_sb is not None:
                    cache, K, col0, Nn = kxn_cache_sb

                    def kxn_producer(nc_, md):
                        n0 = col0 + md.n_tile_idx * md.n_tile
                        return cache[:, bts(md.k_tile_idx, md.k_subtiles),
                                     n0:n0 + md.n_tile]

                    kxn_shape = ShapeInfo(pdims=((128, K // 128),),
                                          fdims=(Nn,))
                else:
                    kxn_producer, kxn_shape = dma_from_dram_kxn(
                        kxn_pool, kxn_ap)
                consumer = dma_to_dram_mxn(mxn_ap)
                if accum_ap is not None:
                    consumer = accumulate_dma_from_dram_mxn(
                        consumer, kxm_pool, accum_ap)
                if post is not None:
                    orig = consumer

                    def consumer(nc_, sbuf, md, orig=orig):
                        post(nc_, sbuf[:, :, :md.n_slice_size], md)
                        orig(nc_, sbuf, md)

                composable_matmul_tile_kernel(
                    tc=tc, kxm_shape=kxm_shape, kxn_shape=kxn_shape,
                    output_type=mxn_ap.dtype, kxm_producer=kxm_producer,
                    kxn_producer=kxn_producer, mxn_consumer=consumer,
                    mxn_subtile_reducer=reducer, psum_n_bufs=psum_bufs)

            def diag_fix(sbuf3, base, md):
                # sbuf3: [p, m_subtiles, n_slice]; absolute col = base + tile
                n0 = base + md.n_tile_idx * md.n_tile
                nsl = sbuf3.shape[-1]
                oms = om_sb[:, :, n0:n0 + nsl]
                dms = dm_sb[:, :, n0:n0 + nsl]
                nc.vector.tensor_tensor(sbuf3[:], sbuf3[:], oms, ALU.mult)
                nc.vector.tensor_tensor(sbuf3[:], sbuf3[:], dms, ALU.add)

            def ag(src, dst):
                nc.gpsimd.collective_compute(
                    "AllGather", ALU.bypass, replica_groups=RG,
                    ins=[src.ap().opt()], outs=[dst.ap().opt()])

            def a2a_slice(srcs, src_rows, ain, dst):
                # srcs: NCH chunk tensors [src_rows, CW] forming a
                # [src_rows, HR] row-shard; dst [8*src_rows, W] = the full
                # matrix's column block owned by this core.
                bw = CW // W  # column blocks per chunk
                for b in range(NCORES):
                    nc.sync.dma_start(
                        ain.ap()[b * src_rows:(b + 1) * src_rows, :],
                        srcs[b // bw].ap()[:, (b % bw) * W:(b % bw + 1) * W])
                nc.gpsimd.collective_compute(
                    "AllToAll", ALU.bypass, replica_groups=RG,
                    ins=[ain.ap().opt()], outs=[dst.ap().opt()])

            AJ_t = [a.ap().rearrange("(s p) n -> p s n", p=128) for a in AJ]

            def adjt_post(nc_, sbuf, md):
                # sbuf: f32 [128, 2, n_slice]; write bf16 diag-fixed copy
                # into the chunk tensor AJ[k] (S2 n_tile == CW)
                nsl = sbuf.shape[-1]
                k = md.n_tile_idx
                bft = aux.tile([128, W // 128, CW], bf16, tag="ajbf")
                nc_.vector.tensor_copy(bft[:, :, :nsl], sbuf[:])
                diag_fix(bft[:, :, :nsl], 0, md)
                nc_.sync.dma_start(AJ_t[k][:, :, :nsl], bft[:, :, :nsl])

            # S1: Mc = U.T @ netouts_c     [LR, W]
            mmk(U_in.ap(), NOc.ap(), Mc.ap())

            # S2: adjT rows = |Mc.T @ aT|  [W, HR]  (f32 out, bf16 chunked
            # copies; AG1 chunk k fires as soon as AJ[k] is complete)
            mmk(Mc.ap(), AT.ap(), ADJT_OUT.ap(), reducer=dve_abs,
                post=adjt_post)
            for k in range(NCH):
                ag(AJ[k], BfC[k])

            # Bcs = B[:, c*W:(c+1)*W] via AllToAll of row-shard blocks
            a2a_slice(AJ, W, BAin, Bcs)

            # SBUF-resident copy of B, reused as kxn in S3/S5/S8;
            # filled per chunk as the AG1 chunks land
            bf_sb = const.tile([128, HR // 128, HR], bf16)
            for k in range(NCH):
                nc.sync.dma_start(
                    bf_sb[:, :, k * CW:(k + 1) * CW],
                    BfC[k].ap().rearrange("(ko p) n -> p ko n", p=128))

            def bf_cache(k):
                return (bf_sb, HR, k * CW, CW)

            # S3: Z rows = |Bcs.T @ B|     [W, HR]  (bf16, diag fixed)
            for k in range(NCH):
                def z_post(nc_, sbuf, md, k=k):
                    diag_fix(sbuf, k * CW, md)
                mmk(Bcs.ap(), None, ZcC[k].ap(), reducer=dve_abs,
                    post=z_post, kxn_cache_sb=bf_cache(k))
                ag(ZcC[k], ZfC[k])

            # S4: T1 = Zf.T @ gc1c         [HR, WH]  (row chunk per Zf chunk)
            for k in range(NCH):
                mmk(ZfC[k].ap(), GC1c.ap(),
                    T1c.ap()[k * CW:(k + 1) * CW, :])

            # S5: h1T rows = relu(T1.T @ B) [WH, HR]
            for k in range(NCH):
                mmk(T1c.ap(), None, H1cC[k].ap(), reducer=dve_relu,
                    kxn_cache_sb=bf_cache(k))

            # H1cs = h1T full [:, c*W:(c+1)*W] (A2A; no AllGather of h1T is
            # needed — its only consumer is this column slice)
            a2a_slice(H1cC, WH, HAin, H1cs)

            # S6: T2 rows = H1cs.T @ gc2   [W, HR]
            for k in range(NCH):
                mmk(H1cs.ap(), GC2.ap()[:, k * CW:(k + 1) * CW],
                    T2cC[k].ap())
                ag(T2cC[k], T2fC[k])

            # T2cs = T2f[:, c*W:(c+1)*W]
            a2a_slice(T2cC, W, TAin, T2cs)

            # S7: X = 0.5*relu(Bcs.T @ T2f)   [W, HR] f32 (h2 rows, halved)
            for k in range(NCH):
                mmk(Bcs.ap(), T2fC[k].ap(),
                    Xc.ap()[:, k * CW:(k + 1) * CW], reducer=dve_relu_half)

            # S8: z rows = 0.5*relu(T2cs.T @ B) + X   [W, HR] f32
            for k in range(NCH):
                mmk(T2cs.ap(), None, Z_OUT.ap()[:, k * CW:(k + 1) * CW],
                    reducer=dve_relu_half,
                    accum_ap=Xc.ap()[:, k * CW:(k + 1) * CW],
                    kxn_cache_sb=bf_cache(k))

    nc.compile()
    return nc


def _get_nc():
    if "nc" not in _CACHE:
        _CACHE["nc"] = _build_nc()
    return _CACHE["nc"]


def _make_in_maps(U, net_outs, gsr_w, gc1_w, gc2_w):
    import ml_dtypes
    bf = ml_dtypes.bfloat16

    aT = np.ascontiguousarray((gsr_w[:, :LR] + gsr_w[:, LR:]).T).astype(bf)
    U_bf = U.astype(bf)
    gc2_bf = gc2_w.astype(bf)

    in_maps = []
    for c in range(NCORES):
        dmask = np.zeros((W, HR), np.float32)
        dmask[np.arange(W), c * W + np.arange(W)] = 1.0
        in_maps.append({
            "u": U_bf,
            "netouts_c": np.ascontiguousarray(
                net_outs[:, c * W:(c + 1) * W]).astype(bf),
            "at": aT,
            "gc1c": np.ascontiguousarray(
                gc1_w[:, c * WH:(c + 1) * WH]).astype(bf),
            "gc2": gc2_bf,
            "dmask": dmask.astype(bf),
        })
    return in_maps


def kernel(lr, gsr_w, start_w, start_b, down_w, down_b, pool_w, pool_b,
           bottom_w, bottom_b, end_w, end_b, up_w, up_b, gc1_w, gc2_w,
           lr_dim, hr_dim):
    global LAST_EXEC_NS
    from concourse.bass_utils import run_bass_kernel_spmd

    net_outs, start_outs, U, _A = _host_prefix(
        lr, start_w, start_b, down_w, down_b, pool_w, pool_b,
        bottom_w, bottom_b, up_w, up_b, end_w, end_b)

    nc = _get_nc()
    in_maps = _make_in_maps(U, net_outs, gsr_w, gc1_w, gc2_w)
    res = run_bass_kernel_spmd(nc, in_maps, list(range(NCORES)), trace=TRACE)
    LAST_EXEC_NS = res.exec_time_ns

    adjT = np.concatenate([res.results[c]["adjt"] for c in range(NCORES)], 0)
    z = np.concatenate([res.results[c]["zrows"] for c in range(NCORES)], 0)
    di = np.arange(HR)
    adj = np.ascontiguousarray(adjT.T)
    adj[di, di] = 1.0
    z[di, di] = 1.0
    return (z.astype(np.float32), net_outs.astype(np.float32),
            start_outs.astype(np.float32), adj.astype(np.float32))


# revision 24
# speedup vs baseline: 1.0033x; 1.0033x over previous
"""AGSRNet Trainium2 kernel.

Host (CPU, exact mirror of the reference for bit-identical top_k / eigh):
  - adjacency normalization, graph U-Net (-> net_outs, start_outs), eigh(A) -> U
Device (8 NeuronCores, one SPMD Bass launch, tensor-parallel over hr columns):
  - M = U.T @ net_outs            (column-sharded)
  - adjT rows = |M_c.T @ a.T|     (+ diag=1)       -> AllGather -> B = adj.T
  - Z rows   = (B[:,cs]).T @ B    (|.|, diag=1)    -> AllGather -> Zf
  - T1 = Zf.T @ gc1[:,cs]
  - h1T rows = relu(T1.T @ B)                      -> AllGather -> H1f
  - T2 rows  = (H1f[:,cs]).T @ gc2                 -> AllGather -> T2f
  - X = 0.5*relu((B[:,cs]).T @ T2f)   (h2 rows, halved)
  - z rows   = 0.5*relu((T2f[:,cs]).T @ B) + X     (diag fixed on host)
All device matmuls run in bf16 with fp32 accumulation.
"""

import numpy as np

LR = 1024
HR = 2048
HID = 1024
NCORES = 8
W = HR // NCORES          # 256 columns of the hr dimension per core
WH = HID // NCORES        # 128 columns of the hidden dim per core

KS = [0.9, 0.7, 0.6, 0.5]

_CACHE = {}

TRACE = False
LAST_EXEC_NS = None


# --------------------------------------------------------------------------
# Host prefix: exact eager-jax-on-CPU mirror of the reference up to net_outs,
# plus eigh(A).  Must follow the reference ops verbatim so that top_k index
# selection and eigenvector signs match the oracle bit-for-bit.
# --------------------------------------------------------------------------
def _host_prefix(lr, start_w, start_b, down_w, down_b, pool_w, pool_b,
                 bottom_w, bottom_b, up_w, up_b, end_w, end_b):
    import jax
    import jax.numpy as jnp

    cpu = jax.devices("cpu")[0]
    with jax.default_device(cpu):
        lr = jnp.asarray(lr)
        n = lr.shape[0]
        r = lr.sum(1) ** -0.5
        r = jnp.where(jnp.isinf(r), 0.0, r)
        A = (lr * r[None, :]).T * r[None, :]
        X = jnp.eye(n, dtype=lr.dtype)

        def _gcn(Ai, X, Wm, b):
            return (Ai @ X) @ Wm + b

        X = _gcn(A, X, jnp.asarray(start_w), jnp.asarray(start_b))
        start_outs = X
        org_X = X
        adj_ms, idxs, downs = [], [], []
        Ai = A
        for i in range(4):
            X = _gcn(Ai, X, jnp.asarray(down_w[i]), jnp.asarray(down_b[i]))
            adj_ms.append(Ai)
            downs.append(X)
            scores = jax.nn.sigmoid(
                (X @ jnp.asarray(pool_w[i]) + jnp.asarray(pool_b[i])) / 100.0)
            k = int(KS[i] * Ai.shape[0])
            vals, idx = jax.lax.top_k(scores, k)
            X = X[idx] * vals[:, None]
            Ai = Ai[idx][:, idx]
            idxs.append(idx)
        X = _gcn(Ai, X, jnp.asarray(bottom_w), jnp.asarray(bottom_b))
        for i in range(4):
            j = 3 - i
            Aj, idx = adj_ms[j], idxs[j]
            Xu = jnp.zeros((Aj.shape[0], X.shape[1]), X.dtype).at[idx].set(X)
            X = _gcn(Aj, Xu, jnp.asarray(up_w[i]), jnp.asarray(up_b[i])) + downs[j]
        X = jnp.concatenate([X, org_X], axis=1)
        net_outs = _gcn(A, X, jnp.asarray(end_w), jnp.asarray(end_b))

        _, U = jnp.linalg.eigh(A, UPLO='U', symmetrize_input=False)

        return (np.asarray(net_outs), np.asarray(start_outs), np.asarray(U),
                np.asarray(A))


# --------------------------------------------------------------------------
# Device graph
# --------------------------------------------------------------------------
def _build_nc():
    import concourse.bass as bass
    import concourse.mybir as mybir
    import concourse.tile as tile
    from concourse import bacc
    from concourse.bass import ts as bts
    from concourse.kernels.tile_matmul import (
        composable_matmul_tile_kernel, dma_from_dram_kxm, dma_from_dram_kxn,
        dma_to_dram_mxn, accumulate_dma_from_dram_mxn, ShapeInfo)

    f32 = mybir.dt.float32
    bf16 = mybir.dt.bfloat16
    i32 = mybir.dt.int32
    AF = mybir.ActivationFunctionType
    ALU = mybir.AluOpType

    nc = bacc.Bacc("TRN2", target_bir_lowering=False, debug=False,
                   num_devices=NCORES)

    # ---- external I/O (per-core) ----
    U_in = nc.dram_tensor("u", [LR, LR], bf16, kind="ExternalInput")
    NOc = nc.dram_tensor("netouts_c", [LR, W], bf16, kind="ExternalInput")
    AT = nc.dram_tensor("at", [LR, HR], bf16, kind="ExternalInput")
    GC1c = nc.dram_tensor("gc1c", [HR, WH], bf16, kind="ExternalInput")
    GC2 = nc.dram_tensor("gc2", [HID, HR], bf16, kind="ExternalInput")
    DM = nc.dram_tensor("dmask", [W, HR], bf16, kind="ExternalInput")

    ADJT_OUT = nc.dram_tensor("adjt", [W, HR], f32, kind="ExternalOutput")
    Z_OUT = nc.dram_tensor("zrows", [W, HR], f32, kind="ExternalOutput")

    # ---- internal DRAM ----
    Mc = nc.dram_tensor("Mc", [LR, W], bf16)
    AJc = nc.dram_tensor("AJc", [W, HR], bf16)
    Bf = nc.dram_tensor("Bf", [HR, HR], bf16, addr_space="Shared")
    BAin = nc.dram_tensor("BAin", [HR, W], bf16)
    Bcs = nc.dram_tensor("Bcs", [HR, W], bf16)
    Zc = nc.dram_tensor("Zc", [W, HR], bf16)
    Zf = nc.dram_tensor("Zf", [HR, HR], bf16, addr_space="Shared")
    T1c = nc.dram_tensor("T1c", [HR, WH], bf16)
    H1c = nc.dram_tensor("H1c", [WH, HR], bf16)
    HAin = nc.dram_tensor("HAin", [HID, W], bf16)
    H1cs = nc.dram_tensor("H1cs", [HID, W], bf16)
    T2c = nc.dram_tensor("T2c", [W, HR], bf16)
    T2f = nc.dram_tensor("T2f", [HR, HR], bf16, addr_space="Shared")
    TAin = nc.dram_tensor("TAin", [HR, W], bf16)
    T2cs = nc.dram_tensor("T2cs", [HR, W], bf16)
    Xc = nc.dram_tensor("Xc", [W, HR], f32)

    RG = [list(range(NCORES))]

    with tile.TileContext(nc) as tc:
        with (
            tc.tile_pool(name="const", bufs=1) as const,
            tc.tile_pool(name="aux", bufs=3) as aux,
            tc.tile_pool(name="kxm", bufs=6) as kxm_pool,
            tc.tile_pool(name="kxn", bufs=6) as kxn_pool,
        ):
            # zero bias for activations
            zbias = const.tile([128, 1], f32)
            nc.any.memset(zbias[:], 0.0)

            # masks resident in SBUF: D and OM = 1 - D, as [128, 2, HR]
            dm_sb = const.tile([128, W // 128, HR], bf16)
            nc.sync.dma_start(
                dm_sb[:], DM.ap().rearrange("(s p) n -> p s n", p=128))
            om_sb = const.tile([128, W // 128, HR], bf16)
            nc.vector.tensor_scalar(om_sb[:], dm_sb[:], -1.0, 1.0,
                                    ALU.mult, ALU.add)

            # PSUM -> SBUF evictions on the vector engine (DVE, ~4x faster
            # than ACT activation copies)
            def dve_copy(nc_, psum, sbuf, md):
                nc_.vector.tensor_copy(sbuf[:], psum[:])

            def dve_abs(nc_, psum, sbuf, md):
                # |x| = max(x, -x): negate into sbuf, then max with psum
                nc_.vector.tensor_scalar(sbuf[:], psum[:], -1.0, None,
                                         ALU.mult)
                nc_.vector.tensor_tensor(sbuf[:], sbuf[:], psum[:], ALU.max)

            def dve_relu(nc_, psum, sbuf, md):
                nc_.vector.tensor_scalar(sbuf[:], psum[:], 0.0, None, ALU.max)

            def dve_relu_half(nc_, psum, sbuf, md):
                nc_.vector.tensor_scalar(sbuf[:], psum[:], 0.0, 0.5,
                                         ALU.max, ALU.mult)

            def mmk(kxm_ap, kxn_ap, mxn_ap, reducer=dve_copy, post=None,
                    accum_ap=None, kxn_cache_sb=None, psum_bufs=2,
                    kxn_producer_shape=None):
                kxm_producer, kxm_shape = dma_from_dram_kxm(kxm_pool, kxm_ap)
                if kxn_producer_shape is not None:
                    kxn_producer, kxn_shape = kxn_producer_shape
                elif kxn_cache_sb is not None:
                    cache, K, col0, Nn = kxn_cache_sb

                    def kxn_producer(nc_, md):
                        n0 = col0 + md.n_tile_idx * md.n_tile
                        return cache[:, bts(md.k_tile_idx, md.k_subtiles),
                                     n0:n0 + md.n_tile]

                    kxn_shape = ShapeInfo(pdims=((128, K // 128),),
                                          fdims=(Nn,))
                else:
                    kxn_producer, kxn_shape = dma_from_dram_kxn(
                        kxn_pool, kxn_ap)
                consumer = dma_to_dram_mxn(mxn_ap)
                if accum_ap is not None:
                    consumer = accumulate_dma_from_dram_mxn(
                        consumer, kxm_pool, accum_ap)
                if post is not None:
                    orig = consumer

                    def consumer(nc_, sbuf, md, orig=orig):
                        post(nc_, sbuf[:, :, :md.n_slice_size], md)
                        orig(nc_, sbuf, md)

                composable_matmul_tile_kernel(
                    tc=tc, kxm_shape=kxm_shape, kxn_shape=kxn_shape,
                    output_type=mxn_ap.dtype, kxm_producer=kxm_producer,
                    kxn_producer=kxn_producer, mxn_consumer=consumer,
                    mxn_subtile_reducer=reducer, psum_n_bufs=psum_bufs)

            def diag_fix(sbuf3, base, md):
                # sbuf3: [p, m_subtiles, n_slice]; absolute col = base + tile
                n0 = base + md.n_tile_idx * md.n_tile
                nsl = sbuf3.shape[-1]
                oms = om_sb[:, :, n0:n0 + nsl]
                dms = dm_sb[:, :, n0:n0 + nsl]
                nc.vector.tensor_tensor(sbuf3[:], sbuf3[:], oms, ALU.mult)
                nc.vector.tensor_tensor(sbuf3[:], sbuf3[:], dms, ALU.add)

            def ag(src, dst):
                nc.gpsimd.collective_compute(
                    "AllGather", ALU.bypass, replica_groups=RG,
                    ins=[src.ap().opt()], outs=[dst.ap().opt()])

            def a2a_slice(src, src_rows, ain, dst):
                # src [src_rows, HR] row-shard; dst [8*src_rows, W] = the
                # full matrix's column block owned by this core.
                for b in range(NCORES):
                    nc.sync.dma_start(
                        ain.ap()[b * src_rows:(b + 1) * src_rows, :],
                        src.ap()[:, b * W:(b + 1) * W])
                nc.gpsimd.collective_compute(
                    "AllToAll", ALU.bypass, replica_groups=RG,
                    ins=[ain.ap().opt()], outs=[dst.ap().opt()])

            AJc_t = AJc.ap().rearrange("(s p) n -> p s n", p=128)

            def adjt_post(nc_, sbuf, md):
                # sbuf: f32 [128, 2, n_slice]; write bf16 diag-fixed copy
                nsl = sbuf.shape[-1]
                n0 = md.n_tile_idx * md.n_tile
                bft = aux.tile([128, W // 128, 512], bf16, tag="ajbf")
                nc_.vector.tensor_copy(bft[:, :, :nsl], sbuf[:])
                diag_fix(bft[:, :, :nsl], 0, md)
                nc_.sync.dma_start(AJc_t[:, :, n0:n0 + nsl], bft[:, :, :nsl])

            # S1: Mc = U.T @ netouts_c     [LR, W]
            mmk(U_in.ap(), NOc.ap(), Mc.ap())

            # S2: adjT rows = |Mc.T @ aT|  [W, HR]  (f32 out, bf16 fixed copy)
            mmk(Mc.ap(), AT.ap(), ADJT_OUT.ap(), reducer=dve_abs,
                post=adjt_post)

            # Bcs = B[:, c*W:(c+1)*W] via AllToAll of row-shard blocks
            a2a_slice(AJc, W, BAin, Bcs)

            # AG1: B = allgather(AJc)      [HR, HR] = adj.T
            ag(AJc, Bf)

            # SBUF-resident copy of B: filled on first use (S3's kxn
            # producer DMAs each tile once), reused by S5 and S8.
            bf_sb = const.tile([128, HR // 128, HR], bf16)
            Bf_t = Bf.ap().rearrange("(ko p) n -> p ko n", p=128)

            def bf_fill_producer(nc_, md):
                n0 = md.n_tile_idx * md.n_tile
                ksl = bts(md.k_tile_idx, md.k_subtiles)
                sl = bf_sb[:, ksl, n0:n0 + md.n_tile]
                nc_.sync.dma_start(sl, Bf_t[:, ksl, n0:n0 + md.n_tile])
                return sl

            bf_shape = ShapeInfo(pdims=((128, HR // 128),), fdims=(HR,))
            bf_cache = (bf_sb, HR, 0, HR)

            # S3: Z rows = |Bcs.T @ B|     [W, HR]  (bf16, diag fixed)
            def z_post(nc_, sbuf, md):
                diag_fix(sbuf, 0, md)

            mmk(Bcs.ap(), None, Zc.ap(), reducer=dve_abs, post=z_post,
                kxn_producer_shape=(bf_fill_producer, bf_shape))

            # AG2: Zf = allgather(Zc)      [HR, HR]
            ag(Zc, Zf)

            # S4: T1 = Zf.T @ gc1c         [HR, WH]
            mmk(Zf.ap(), GC1c.ap(), T1c.ap())

            # S5: h1T rows = relu(T1.T @ B) [WH, HR]
            mmk(T1c.ap(), None, H1c.ap(), reducer=dve_relu,
                kxn_cache_sb=bf_cache)

            # H1cs = h1T full [:, c*W:(c+1)*W] (A2A; no AllGather of h1T is
            # needed — its only consumer is this column slice)
            a2a_slice(H1c, WH, HAin, H1cs)

            # S6: T2 rows = H1cs.T @ gc2   [W, HR]
            mmk(H1cs.ap(), GC2.ap(), T2c.ap())

            # AG4: T2f = allgather(T2c)    [HR, HR]
            ag(T2c, T2f)

            # T2cs = T2f[:, c*W:(c+1)*W]
            a2a_slice(T2c, W, TAin, T2cs)

            # S7: X = 0.5*relu(Bcs.T @ T2f)   [W, HR] f32 (h2 rows, halved)
            mmk(Bcs.ap(), T2f.ap(), Xc.ap(), reducer=dve_relu_half)

            # S8: z rows = 0.5*relu(T2cs.T @ B) + X   [W, HR] f32
            mmk(T2cs.ap(), None, Z_OUT.ap(), reducer=dve_relu_half,
                accum_ap=Xc.ap(), kxn_cache_sb=bf_cache)

    nc.compile()
    return nc


def _get_nc():
    if "nc" not in _CACHE:
        _CACHE["nc"] = _build_nc()
    return _CACHE["nc"]


def _make_in_maps(U, net_outs, gsr_w, gc1_w, gc2_w):
    import ml_dtypes
    bf = ml_dtypes.bfloat16

    aT = np.ascontiguousarray((gsr_w[:, :LR] + gsr_w[:, LR:]).T).astype(bf)
    U_bf = U.astype(bf)
    gc2_bf = gc2_w.astype(bf)

    in_maps = []
    for c in range(NCORES):
        dmask = np.zeros((W, HR), np.float32)
        dmask[np.arange(W), c * W + np.arange(W)] = 1.0
        in_maps.append({
            "u": U_bf,
            "netouts_c": np.ascontiguousarray(
                net_outs[:, c * W:(c + 1) * W]).astype(bf),
            "at": aT,
            "gc1c": np.ascontiguousarray(
                gc1_w[:, c * WH:(c + 1) * WH]).astype(bf),
            "gc2": gc2_bf,
            "dmask": dmask.astype(bf),
        })
    return in_maps


def kernel(lr, gsr_w, start_w, start_b, down_w, down_b, pool_w, pool_b,
           bottom_w, bottom_b, end_w, end_b, up_w, up_b, gc1_w, gc2_w,
           lr_dim, hr_dim):
    global LAST_EXEC_NS
    from concourse.bass_utils import run_bass_kernel_spmd

    net_outs, start_outs, U, _A = _host_prefix(
        lr, start_w, start_b, down_w, down_b, pool_w, pool_b,
        bottom_w, bottom_b, up_w, up_b, end_w, end_b)

    nc = _get_nc()
    in_maps = _make_in_maps(U, net_outs, gsr_w, gc1_w, gc2_w)
    res = run_bass_kernel_spmd(nc, in_maps, list(range(NCORES)), trace=TRACE)
    LAST_EXEC_NS = res.exec_time_ns

    adjT = np.concatenate([res.results[c]["adjt"] for c in range(NCORES)], 0)
    z = np.concatenate([res.results[c]["zrows"] for c in range(NCORES)], 0)
    di = np.arange(HR)
    adj = np.ascontiguousarray(adjT.T)
    adj[di, di] = 1.0
    z[di, di] = 1.0
    return (z.astype(np.float32), net_outs.astype(np.float32),
            start_outs.astype(np.float32), adj.astype(np.float32))


# revision 33
# speedup vs baseline: 1.1496x; 1.1458x over previous
"""AGSRNet Trainium2 kernel.

Host (CPU, exact mirror of the reference for bit-identical top_k / eigh):
  - adjacency normalization, graph U-Net (-> net_outs, start_outs), eigh(A) -> U
Device (8 NeuronCores, one SPMD Bass launch, tensor-parallel over hr columns):
  - M = U.T @ net_outs            (column-sharded)
  - adjT rows = |M_c.T @ a.T|     (+ diag=1)       -> AllGather -> B = adj.T
  - Z rows   = (B[:,cs]).T @ B    (|.|, diag=1)    -> AllGather -> Zf
  - T1 = Zf.T @ gc1[:,cs]
  - h1T rows = relu(T1.T @ B)                      -> AllGather -> H1f
  - T2 rows  = (H1f[:,cs]).T @ gc2                 -> AllGather -> T2f
  - X = 0.5*relu((B[:,cs]).T @ T2f)   (h2 rows, halved)
  - z rows   = 0.5*relu((T2f[:,cs]).T @ B) + X     (diag fixed on host)
All device matmuls run in bf16 with fp32 accumulation.
"""

import numpy as np

LR = 1024
HR = 2048
HID = 1024
NCORES = 8
W = HR // NCORES          # 256 columns of the hr dimension per core
WH = HID // NCORES        # 128 columns of the hidden dim per core

KS = [0.9, 0.7, 0.6, 0.5]

_CACHE = {}

TRACE = False
LAST_EXEC_NS = None


# --------------------------------------------------------------------------
# Host prefix: exact eager-jax-on-CPU mirror of the reference up to net_outs,
# plus eigh(A).  Must follow the reference ops verbatim so that top_k index
# selection and eigenvector signs match the oracle bit-for-bit.
# --------------------------------------------------------------------------
def _host_prefix(lr, start_w, start_b, down_w, down_b, pool_w, pool_b,
                 bottom_w, bottom_b, up_w, up_b, end_w, end_b):
    import jax
    import jax.numpy as jnp

    cpu = jax.devices("cpu")[0]
    with jax.default_device(cpu):
        lr = jnp.asarray(lr)
        n = lr.shape[0]
        r = lr.sum(1) ** -0.5
        r = jnp.where(jnp.isinf(r), 0.0, r)
        A = (lr * r[None, :]).T * r[None, :]
        X = jnp.eye(n, dtype=lr.dtype)

        def _gcn(Ai, X, Wm, b):
            return (Ai @ X) @ Wm + b

        X = _gcn(A, X, jnp.asarray(start_w), jnp.asarray(start_b))
        start_outs = X
        org_X = X
        adj_ms, idxs, downs = [], [], []
        Ai = A
        for i in range(4):
            X = _gcn(Ai, X, jnp.asarray(down_w[i]), jnp.asarray(down_b[i]))
            adj_ms.append(Ai)
            downs.append(X)
            scores = jax.nn.sigmoid(
                (X @ jnp.asarray(pool_w[i]) + jnp.asarray(pool_b[i])) / 100.0)
            k = int(KS[i] * Ai.shape[0])
            vals, idx = jax.lax.top_k(scores, k)
            X = X[idx] * vals[:, None]
            Ai = Ai[idx][:, idx]
            idxs.append(idx)
        X = _gcn(Ai, X, jnp.asarray(bottom_w), jnp.asarray(bottom_b))
        for i in range(4):
            j = 3 - i
            Aj, idx = adj_ms[j], idxs[j]
            Xu = jnp.zeros((Aj.shape[0], X.shape[1]), X.dtype).at[idx].set(X)
            X = _gcn(Aj, Xu, jnp.asarray(up_w[i]), jnp.asarray(up_b[i])) + downs[j]
        X = jnp.concatenate([X, org_X], axis=1)
        net_outs = _gcn(A, X, jnp.asarray(end_w), jnp.asarray(end_b))

        _, U = jnp.linalg.eigh(A, UPLO='U', symmetrize_input=False)

        return (np.asarray(net_outs), np.asarray(start_outs), np.asarray(U),
                np.asarray(A))


# --------------------------------------------------------------------------
# Device graph
# --------------------------------------------------------------------------
def _build_nc():
    import concourse.bass as bass
    import concourse.mybir as mybir
    import concourse.tile as tile
    from concourse import bacc
    from concourse.bass import ts as bts
    from concourse.kernels.tile_matmul import (
        composable_matmul_tile_kernel, dma_from_dram_kxm, dma_from_dram_kxn,
        dma_to_dram_mxn, accumulate_dma_from_dram_mxn, ShapeInfo)

    f32 = mybir.dt.float32
    bf16 = mybir.dt.bfloat16
    i32 = mybir.dt.int32
    AF = mybir.ActivationFunctionType
    ALU = mybir.AluOpType

    nc = bacc.Bacc("TRN2", target_bir_lowering=False, debug=False,
                   num_devices=NCORES)

    # ---- external I/O (per-core) ----
    U_in = nc.dram_tensor("u", [LR, LR], bf16, kind="ExternalInput")
    NOc = nc.dram_tensor("netouts_c", [LR, W], bf16, kind="ExternalInput")
    AT = nc.dram_tensor("at", [LR, HR], bf16, kind="ExternalInput")
    GC1c = nc.dram_tensor("gc1c", [HR, WH], bf16, kind="ExternalInput")
    GC2 = nc.dram_tensor("gc2", [HID, HR], bf16, kind="ExternalInput")
    DM = nc.dram_tensor("dmask", [W, HR], bf16, kind="ExternalInput")

    ADJT_OUT = nc.dram_tensor("adjt", [W, HR], f32, kind="ExternalOutput")
    Z_OUT = nc.dram_tensor("zrows", [W, HR], f32, kind="ExternalOutput")

    # ---- internal DRAM ----
    Mc = nc.dram_tensor("Mc", [LR, W], bf16)
    AJc = nc.dram_tensor("AJc", [W, HR], bf16)
    Bf = nc.dram_tensor("Bf", [HR, HR], bf16, addr_space="Shared")
    BAin = nc.dram_tensor("BAin", [HR, W], bf16)
    Bcs = nc.dram_tensor("Bcs", [HR, W], bf16)
    Zc = nc.dram_tensor("Zc", [W, HR], bf16)
    Zf = nc.dram_tensor("Zf", [HR, HR], bf16, addr_space="Shared")
    T1c = nc.dram_tensor("T1c", [HR, WH], bf16)
    H1c = nc.dram_tensor("H1c", [WH, HR], bf16)
    HAin = nc.dram_tensor("HAin", [HID, W], bf16)
    H1cs = nc.dram_tensor("H1cs", [HID, W], bf16)
    T2c = nc.dram_tensor("T2c", [W, HR], bf16)
    T2f = nc.dram_tensor("T2f", [HR, HR], bf16, addr_space="Shared")
    TAin = nc.dram_tensor("TAin", [HR, W], bf16)
    T2cs = nc.dram_tensor("T2cs", [HR, W], bf16)
    Xc = nc.dram_tensor("Xc", [W, HR], f32)

    RG = [list(range(NCORES))]

    with tile.TileContext(nc) as tc:
        with (
            tc.tile_pool(name="const", bufs=1) as const,
            tc.tile_pool(name="aux", bufs=3) as aux,
            tc.tile_pool(name="kxm", bufs=5) as kxm_pool,
            tc.tile_pool(name="kxn", bufs=5) as kxn_pool,
        ):
            # zero bias for activations
            zbias = const.tile([128, 1], f32)
            nc.any.memset(zbias[:], 0.0)

            # diag mask resident in SBUF as [128, 2, HR]
            dm_sb = const.tile([128, W // 128, HR], bf16)
            nc.sync.dma_start(
                dm_sb[:], DM.ap().rearrange("(s p) n -> p s n", p=128))

            # PSUM -> SBUF evictions on the vector engine (DVE, ~4x faster
            # than ACT activation copies)
            def dve_copy(nc_, psum, sbuf, md):
                nc_.vector.tensor_copy(sbuf[:], psum[:])

            def dve_abs(nc_, psum, sbuf, md):
                # |x| = max(x, -x): negate into sbuf, then max with psum
                nc_.vector.tensor_scalar(sbuf[:], psum[:], -1.0, None,
                                         ALU.mult)
                nc_.vector.tensor_tensor(sbuf[:], sbuf[:], psum[:], ALU.max)

            def dve_relu(nc_, psum, sbuf, md):
                nc_.vector.tensor_scalar(sbuf[:], psum[:], 0.0, None, ALU.max)

            def dve_relu_half(nc_, psum, sbuf, md):
                nc_.vector.tensor_scalar(sbuf[:], psum[:], 0.0, 0.5,
                                         ALU.max, ALU.mult)

            def mmk(kxm_ap, kxn_ap, mxn_ap, reducer=dve_copy, post=None,
                    accum_ap=None, kxn_cache_sb=None, psum_bufs=2,
                    kxn_producer_shape=None):
                kxm_producer, kxm_shape = dma_from_dram_kxm(kxm_pool, kxm_ap)
                if kxn_producer_shape is not None:
                    kxn_producer, kxn_shape = kxn_producer_shape
                elif kxn_cache_sb is not None:
                    cache, K, col0, Nn = kxn_cache_sb

                    def kxn_producer(nc_, md):
                        n0 = col0 + md.n_tile_idx * md.n_tile
                        return cache[:, bts(md.k_tile_idx, md.k_subtiles),
                                     n0:n0 + md.n_tile]

                    kxn_shape = ShapeInfo(pdims=((128, K // 128),),
                                          fdims=(Nn,))
                else:
                    kxn_producer, kxn_shape = dma_from_dram_kxn(
                        kxn_pool, kxn_ap)
                consumer = dma_to_dram_mxn(mxn_ap)
                if accum_ap is not None:
                    consumer = accumulate_dma_from_dram_mxn(
                        consumer, kxm_pool, accum_ap)
                if post is not None:
                    orig = consumer

                    def consumer(nc_, sbuf, md, orig=orig):
                        post(nc_, sbuf[:, :, :md.n_slice_size], md)
                        orig(nc_, sbuf, md)

                composable_matmul_tile_kernel(
                    tc=tc, kxm_shape=kxm_shape, kxn_shape=kxn_shape,
                    output_type=mxn_ap.dtype, kxm_producer=kxm_producer,
                    kxn_producer=kxn_producer, mxn_consumer=consumer,
                    mxn_subtile_reducer=reducer, psum_n_bufs=psum_bufs)

            def diag_fix(sbuf3, base, md):
                # sbuf3: [p, m_subtiles, n_slice]; absolute col = base + tile
                # t <- t*(1-D) + D  ==  t - (t-1)*D
                n0 = base + md.n_tile_idx * md.n_tile
                nsl = sbuf3.shape[-1]
                dms = dm_sb[:, :, n0:n0 + nsl]
                tmp = aux.tile([128, W // 128, 512], bf16, tag="dtmp")
                nc.vector.scalar_tensor_tensor(
                    tmp[:, :, :nsl], sbuf3[:], 1.0, dms,
                    ALU.subtract, ALU.mult)
                nc.vector.tensor_tensor(sbuf3[:], sbuf3[:], tmp[:, :, :nsl],
                                        ALU.subtract)

            def ag(src, dst):
                nc.gpsimd.collective_compute(
                    "AllGather", ALU.bypass, replica_groups=RG,
                    ins=[src.ap().opt()], outs=[dst.ap().opt()])

            def a2a_slice(src, src_rows, ain, dst):
                # src [src_rows, HR] row-shard; dst [8*src_rows, W] = the
                # full matrix's column block owned by this core.
                for b in range(NCORES):
                    nc.sync.dma_start(
                        ain.ap()[b * src_rows:(b + 1) * src_rows, :],
                        src.ap()[:, b * W:(b + 1) * W])
                nc.gpsimd.collective_compute(
                    "AllToAll", ALU.bypass, replica_groups=RG,
                    ins=[ain.ap().opt()], outs=[dst.ap().opt()])

            # Persistent SBUF staging for the row-shard outputs that feed
            # collectives: lets us emit a few large DMAs instead of
            # thousands of 512B strided packets.
            aj_sb = const.tile([128, W // 128, HR], bf16)
            h1_sb = const.tile([128, WH // 128, HR], bf16)
            t2_sb = const.tile([128, W // 128, HR], bf16)

            AJc_t = AJc.ap().rearrange("(s p) n -> p s n", p=128)

            def adjt_post(nc_, sbuf, md):
                # sbuf: f32 [128, 2, n_slice]; bf16 diag-fixed copy into
                # the persistent staging tile
                nsl = sbuf.shape[-1]
                n0 = md.n_tile_idx * md.n_tile
                sl = aj_sb[:, :, n0:n0 + nsl]
                nc_.vector.tensor_copy(sl, sbuf[:])
                diag_fix(sl, 0, md)

            def stage_out(src_sb, rows, dram_rowmajor, a2a_in):
                # src_sb [128, rows//128, HR] -> dram [rows, HR] (one DMA)
                # and the A2A input [8*rows, W] (one DMA per target block)
                if dram_rowmajor is not None:
                    nc.sync.dma_start(
                        dram_rowmajor.ap().rearrange("(s p) n -> p s n",
                                                     p=128),
                        src_sb[:])
                for b in range(NCORES):
                    nc.sync.dma_start(
                        a2a_in.ap()[b * rows:(b + 1) * rows, :].rearrange(
                            "(s p) j -> p s j", p=128),
                        src_sb[:, :, b * W:(b + 1) * W])

            # S1: Mc = U.T @ netouts_c     [LR, W]
            mmk(U_in.ap(), NOc.ap(), Mc.ap())

            # S2: adjT rows = |Mc.T @ aT|  [W, HR]  (f32 out, bf16 fixed copy)
            mmk(Mc.ap(), AT.ap(), ADJT_OUT.ap(), reducer=dve_abs,
                post=adjt_post)
            stage_out(aj_sb, W, AJc, BAin)

            # Bcs = B[:, c*W:(c+1)*W] via AllToAll of row-shard blocks
            nc.gpsimd.collective_compute(
                "AllToAll", ALU.bypass, replica_groups=RG,
                ins=[BAin.ap().opt()], outs=[Bcs.ap().opt()])

            # AG1: B = allgather(AJc)      [HR, HR] = adj.T
            ag(AJc, Bf)

            # SBUF-resident copy of B: filled on first use (S3's kxn
            # producer DMAs each tile once), reused by S5 and S8.
            bf_sb = const.tile([128, HR // 128, HR], bf16)
            Bf_t = Bf.ap().rearrange("(ko p) n -> p ko n", p=128)

            def bf_fill_producer(nc_, md):
                n0 = md.n_tile_idx * md.n_tile
                ksl = bts(md.k_tile_idx, md.k_subtiles)
                sl = bf_sb[:, ksl, n0:n0 + md.n_tile]
                nc_.sync.dma_start(sl, Bf_t[:, ksl, n0:n0 + md.n_tile])
                return sl

            bf_shape = ShapeInfo(pdims=((128, HR // 128),), fdims=(HR,))
            bf_cache = (bf_sb, HR, 0, HR)

            # S3: Z rows = |Bcs.T @ B|     [W, HR]  (bf16, diag fixed)
            def z_post(nc_, sbuf, md):
                diag_fix(sbuf, 0, md)

            mmk(Bcs.ap(), None, Zc.ap(), reducer=dve_abs, post=z_post,
                kxn_producer_shape=(bf_fill_producer, bf_shape))

            # AG2: Zf = allgather(Zc)      [HR, HR]
            ag(Zc, Zf)

            # S4: T1 = Zf.T @ gc1c         [HR, WH]
            mmk(Zf.ap(), GC1c.ap(), T1c.ap())

            # S5: h1T rows = relu(T1.T @ B) [WH, HR]
            def h1_post(nc_, sbuf, md):
                n0 = md.n_tile_idx * md.n_tile
                nc_.vector.tensor_copy(
                    h1_sb[:, :, n0:n0 + sbuf.shape[-1]], sbuf[:])

            mmk(T1c.ap(), None, H1c.ap(), reducer=dve_relu,
                kxn_cache_sb=bf_cache, post=h1_post)

            # H1cs = h1T full [:, c*W:(c+1)*W] (A2A; no AllGather of h1T is
            # needed — its only consumer is this column slice)
            stage_out(h1_sb, WH, None, HAin)
            nc.gpsimd.collective_compute(
                "AllToAll", ALU.bypass, replica_groups=RG,
                ins=[HAin.ap().opt()], outs=[H1cs.ap().opt()])

            # S6: T2 rows = H1cs.T @ gc2   [W, HR]
            def t2_post(nc_, sbuf, md):
                n0 = md.n_tile_idx * md.n_tile
                nc_.vector.tensor_copy(
                    t2_sb[:, :, n0:n0 + sbuf.shape[-1]], sbuf[:])

            mmk(H1cs.ap(), GC2.ap(), T2c.ap(), post=t2_post)
            stage_out(t2_sb, W, None, TAin)

            # T2cs = T2f[:, c*W:(c+1)*W]
            nc.gpsimd.collective_compute(
                "AllToAll", ALU.bypass, replica_groups=RG,
                ins=[TAin.ap().opt()], outs=[T2cs.ap().opt()])

            # AG4: T2f = allgather(T2c)    [HR, HR]
            ag(T2c, T2f)

            # S7: X = 0.5*relu(Bcs.T @ T2f)   [W, HR] f32 (h2 rows, halved)
            mmk(Bcs.ap(), T2f.ap(), Xc.ap(), reducer=dve_relu_half)

            # S8: z rows = 0.5*relu(T2cs.T @ B) + X   [W, HR] f32
            mmk(T2cs.ap(), None, Z_OUT.ap(), reducer=dve_relu_half,
                accum_ap=Xc.ap(), kxn_cache_sb=bf_cache)

    nc.compile()
    return nc


def _get_nc():
    if "nc" not in _CACHE:
        _CACHE["nc"] = _build_nc()
    return _CACHE["nc"]


def _make_in_maps(U, net_outs, gsr_w, gc1_w, gc2_w):
    import ml_dtypes
    bf = ml_dtypes.bfloat16

    aT = np.ascontiguousarray((gsr_w[:, :LR] + gsr_w[:, LR:]).T).astype(bf)
    U_bf = U.astype(bf)
    gc2_bf = gc2_w.astype(bf)

    in_maps = []
    for c in range(NCORES):
        dmask = np.zeros((W, HR), np.float32)
        dmask[np.arange(W), c * W + np.arange(W)] = 1.0
        in_maps.append({
            "u": U_bf,
            "netouts_c": np.ascontiguousarray(
                net_outs[:, c * W:(c + 1) * W]).astype(bf),
            "at": aT,
            "gc1c": np.ascontiguousarray(
                gc1_w[:, c * WH:(c + 1) * WH]).astype(bf),
            "gc2": gc2_bf,
            "dmask": dmask.astype(bf),
        })
    return in_maps


def kernel(lr, gsr_w, start_w, start_b, down_w, down_b, pool_w, pool_b,
           bottom_w, bottom_b, end_w, end_b, up_w, up_b, gc1_w, gc2_w,
           lr_dim, hr_dim):
    global LAST_EXEC_NS
    from concourse.bass_utils import run_bass_kernel_spmd

    net_outs, start_outs, U, _A = _host_prefix(
        lr, start_w, start_b, down_w, down_b, pool_w, pool_b,
        bottom_w, bottom_b, up_w, up_b, end_w, end_b)

    nc = _get_nc()
    in_maps = _make_in_maps(U, net_outs, gsr_w, gc1_w, gc2_w)
    res = run_bass_kernel_spmd(nc, in_maps, list(range(NCORES)), trace=TRACE)
    LAST_EXEC_NS = res.exec_time_ns

    adjT = np.concatenate([res.results[c]["adjt"] for c in range(NCORES)], 0)
    z = np.concatenate([res.results[c]["zrows"] for c in range(NCORES)], 0)
    di = np.arange(HR)
    adj = np.ascontiguousarray(adjT.T)
    adj[di, di] = 1.0
    z[di, di] = 1.0
    return (z.astype(np.float32), net_outs.astype(np.float32),
            start_outs.astype(np.float32), adj.astype(np.float32))


# revision 35
# speedup vs baseline: 1.1769x; 1.0237x over previous
"""AGSRNet Trainium2 kernel.

Host (CPU, exact mirror of the reference for bit-identical top_k / eigh):
  - adjacency normalization, graph U-Net (-> net_outs, start_outs), eigh(A) -> U
Device (8 NeuronCores, one SPMD Bass launch, tensor-parallel over hr columns):
  - M = U.T @ net_outs            (column-sharded)
  - adjT rows = |M_c.T @ a.T|     (+ diag=1)       -> AllGather -> B = adj.T
  - Z rows   = (B[:,cs]).T @ B    (|.|, diag=1)    -> AllGather -> Zf
  - T1 = Zf.T @ gc1[:,cs]
  - h1T rows = relu(T1.T @ B)                      -> AllGather -> H1f
  - T2 rows  = (H1f[:,cs]).T @ gc2                 -> AllGather -> T2f
  - X = 0.5*relu((B[:,cs]).T @ T2f)   (h2 rows, halved)
  - z rows   = 0.5*relu((T2f[:,cs]).T @ B) + X     (diag fixed on host)
All device matmuls run in bf16 with fp32 accumulation.
"""

import numpy as np

LR = 1024
HR = 2048
HID = 1024
NCORES = 8
W = HR // NCORES          # 256 columns of the hr dimension per core
WH = HID // NCORES        # 128 columns of the hidden dim per core

KS = [0.9, 0.7, 0.6, 0.5]

_CACHE = {}

TRACE = False
LAST_EXEC_NS = None


# --------------------------------------------------------------------------
# Host prefix: exact eager-jax-on-CPU mirror of the reference up to net_outs,
# plus eigh(A).  Must follow the reference ops verbatim so that top_k index
# selection and eigenvector signs match the oracle bit-for-bit.
# --------------------------------------------------------------------------
def _host_prefix(lr, start_w, start_b, down_w, down_b, pool_w, pool_b,
                 bottom_w, bottom_b, up_w, up_b, end_w, end_b):
    import jax
    import jax.numpy as jnp

    cpu = jax.devices("cpu")[0]
    with jax.default_device(cpu):
        lr = jnp.asarray(lr)
        n = lr.shape[0]
        r = lr.sum(1) ** -0.5
        r = jnp.where(jnp.isinf(r), 0.0, r)
        A = (lr * r[None, :]).T * r[None, :]
        X = jnp.eye(n, dtype=lr.dtype)

        def _gcn(Ai, X, Wm, b):
            return (Ai @ X) @ Wm + b

        X = _gcn(A, X, jnp.asarray(start_w), jnp.asarray(start_b))
        start_outs = X
        org_X = X
        adj_ms, idxs, downs = [], [], []
        Ai = A
        for i in range(4):
            X = _gcn(Ai, X, jnp.asarray(down_w[i]), jnp.asarray(down_b[i]))
            adj_ms.append(Ai)
            downs.append(X)
            scores = jax.nn.sigmoid(
                (X @ jnp.asarray(pool_w[i]) + jnp.asarray(pool_b[i])) / 100.0)
            k = int(KS[i] * Ai.shape[0])
            vals, idx = jax.lax.top_k(scores, k)
            X = X[idx] * vals[:, None]
            Ai = Ai[idx][:, idx]
            idxs.append(idx)
        X = _gcn(Ai, X, jnp.asarray(bottom_w), jnp.asarray(bottom_b))
        for i in range(4):
            j = 3 - i
            Aj, idx = adj_ms[j], idxs[j]
            Xu = jnp.zeros((Aj.shape[0], X.shape[1]), X.dtype).at[idx].set(X)
            X = _gcn(Aj, Xu, jnp.asarray(up_w[i]), jnp.asarray(up_b[i])) + downs[j]
        X = jnp.concatenate([X, org_X], axis=1)
        net_outs = _gcn(A, X, jnp.asarray(end_w), jnp.asarray(end_b))

        _, U = jnp.linalg.eigh(A, UPLO='U', symmetrize_input=False)

        return (np.asarray(net_outs), np.asarray(start_outs), np.asarray(U),
                np.asarray(A))


# --------------------------------------------------------------------------
# Device graph
# --------------------------------------------------------------------------
def _build_nc():
    import concourse.bass as bass
    import concourse.mybir as mybir
    import concourse.tile as tile
    from concourse import bacc
    from concourse.bass import ts as bts
    from concourse.kernels.tile_matmul import (
        composable_matmul_tile_kernel, dma_from_dram_kxm, dma_from_dram_kxn,
        dma_to_dram_mxn, accumulate_dma_from_dram_mxn, ShapeInfo)

    f32 = mybir.dt.float32
    bf16 = mybir.dt.bfloat16
    i32 = mybir.dt.int32
    AF = mybir.ActivationFunctionType
    ALU = mybir.AluOpType

    nc = bacc.Bacc("TRN2", target_bir_lowering=False, debug=False,
                   num_devices=NCORES)

    # ---- external I/O (per-core) ----
    U_in = nc.dram_tensor("u", [LR, LR], bf16, kind="ExternalInput")
    NOc = nc.dram_tensor("netouts_c", [LR, W], bf16, kind="ExternalInput")
    AT = nc.dram_tensor("at", [LR, HR], bf16, kind="ExternalInput")
    GC1c = nc.dram_tensor("gc1c", [HR, WH], bf16, kind="ExternalInput")
    GC2 = nc.dram_tensor("gc2", [HID, HR], bf16, kind="ExternalInput")
    DM = nc.dram_tensor("dmask", [W, HR], bf16, kind="ExternalInput")

    ADJT_OUT = nc.dram_tensor("adjt", [W, HR], f32, kind="ExternalOutput")
    Z_OUT = nc.dram_tensor("zrows", [W, HR], f32, kind="ExternalOutput")

    # ---- internal DRAM ----
    HH = HR // 2  # AG half width

    Mc = nc.dram_tensor("Mc", [LR, W], bf16)
    AJh = [nc.dram_tensor(f"AJ{h}", [W, HH], bf16) for h in range(2)]
    Bfh = [nc.dram_tensor(f"Bf{h}", [HR, HH], bf16, addr_space="Shared")
           for h in range(2)]
    BAin = nc.dram_tensor("BAin", [HR, W], bf16)
    Bcs = nc.dram_tensor("Bcs", [HR, W], bf16)
    Zch = [nc.dram_tensor(f"Zc{h}", [W, HH], bf16) for h in range(2)]
    Zfh = [nc.dram_tensor(f"Zf{h}", [HR, HH], bf16, addr_space="Shared")
           for h in range(2)]
    T1c = nc.dram_tensor("T1c", [HR, WH], bf16)
    H1c = nc.dram_tensor("H1c", [WH, HR], bf16)
    HAin = nc.dram_tensor("HAin", [HID, W], bf16)
    H1cs = nc.dram_tensor("H1cs", [HID, W], bf16)
    T2ch = [nc.dram_tensor(f"T2c{h}", [W, HH], bf16) for h in range(2)]
    T2fh = [nc.dram_tensor(f"T2f{h}", [HR, HH], bf16, addr_space="Shared")
            for h in range(2)]
    TAin = nc.dram_tensor("TAin", [HR, W], bf16)
    T2cs = nc.dram_tensor("T2cs", [HR, W], bf16)
    Xc = nc.dram_tensor("Xc", [W, HR], f32)

    RG = [list(range(NCORES))]

    with tile.TileContext(nc) as tc:
        with (
            tc.tile_pool(name="const", bufs=1) as const,
            tc.tile_pool(name="aux", bufs=3) as aux,
            tc.tile_pool(name="kxm", bufs=5) as kxm_pool,
            tc.tile_pool(name="kxn", bufs=5) as kxn_pool,
        ):
            # zero bias for activations
            zbias = const.tile([128, 1], f32)
            nc.any.memset(zbias[:], 0.0)

            # diag mask resident in SBUF as [128, 2, HR]
            dm_sb = const.tile([128, W // 128, HR], bf16)
            nc.sync.dma_start(
                dm_sb[:], DM.ap().rearrange("(s p) n -> p s n", p=128))

            # PSUM -> SBUF evictions on the vector engine (DVE, ~4x faster
            # than ACT activation copies)
            def dve_copy(nc_, psum, sbuf, md):
                nc_.vector.tensor_copy(sbuf[:], psum[:])

            def dve_abs(nc_, psum, sbuf, md):
                # |x| = max(x, -x): negate into sbuf, then max with psum
                nc_.vector.tensor_scalar(sbuf[:], psum[:], -1.0, None,
                                         ALU.mult)
                nc_.vector.tensor_tensor(sbuf[:], sbuf[:], psum[:], ALU.max)

            def dve_relu(nc_, psum, sbuf, md):
                nc_.vector.tensor_scalar(sbuf[:], psum[:], 0.0, None, ALU.max)

            def dve_relu_half(nc_, psum, sbuf, md):
                nc_.vector.tensor_scalar(sbuf[:], psum[:], 0.0, 0.5,
                                         ALU.max, ALU.mult)

            def mmk(kxm_ap, kxn_ap, mxn_ap, reducer=dve_copy, post=None,
                    accum_ap=None, kxn_cache_sb=None, psum_bufs=2,
                    kxn_producer_shape=None):
                kxm_producer, kxm_shape = dma_from_dram_kxm(kxm_pool, kxm_ap)
                if kxn_producer_shape is not None:
                    kxn_producer, kxn_shape = kxn_producer_shape
                elif kxn_cache_sb is not None:
                    cache, K, col0, Nn = kxn_cache_sb

                    def kxn_producer(nc_, md):
                        n0 = col0 + md.n_tile_idx * md.n_tile
                        return cache[:, bts(md.k_tile_idx, md.k_subtiles),
                                     n0:n0 + md.n_tile]

                    kxn_shape = ShapeInfo(pdims=((128, K // 128),),
                                          fdims=(Nn,))
                else:
                    kxn_producer, kxn_shape = dma_from_dram_kxn(
                        kxn_pool, kxn_ap)
                consumer = dma_to_dram_mxn(mxn_ap)
                if accum_ap is not None:
                    consumer = accumulate_dma_from_dram_mxn(
                        consumer, kxm_pool, accum_ap)
                if post is not None:
                    orig = consumer

                    def consumer(nc_, sbuf, md, orig=orig):
                        post(nc_, sbuf[:, :, :md.n_slice_size], md)
                        orig(nc_, sbuf, md)

                composable_matmul_tile_kernel(
                    tc=tc, kxm_shape=kxm_shape, kxn_shape=kxn_shape,
                    output_type=mxn_ap.dtype, kxm_producer=kxm_producer,
                    kxn_producer=kxn_producer, mxn_consumer=consumer,
                    mxn_subtile_reducer=reducer, psum_n_bufs=psum_bufs)

            def diag_fix(sbuf3, base, md):
                # sbuf3: [p, m_subtiles, n_slice]; absolute col = base + tile
                # t <- t*(1-D) + D  ==  t - (t-1)*D
                n0 = base + md.n_tile_idx * md.n_tile
                nsl = sbuf3.shape[-1]
                dms = dm_sb[:, :, n0:n0 + nsl]
                tmp = aux.tile([128, W // 128, 512], bf16, tag="dtmp")
                nc.vector.scalar_tensor_tensor(
                    tmp[:, :, :nsl], sbuf3[:], 1.0, dms,
                    ALU.subtract, ALU.mult)
                nc.vector.tensor_tensor(sbuf3[:], sbuf3[:], tmp[:, :, :nsl],
                                        ALU.subtract)

            def ag(src, dst):
                nc.gpsimd.collective_compute(
                    "AllGather", ALU.bypass, replica_groups=RG,
                    ins=[src.ap().opt()], outs=[dst.ap().opt()])

            def a2a_slice(src, src_rows, ain, dst):
                # src [src_rows, HR] row-shard; dst [8*src_rows, W] = the
                # full matrix's column block owned by this core.
                for b in range(NCORES):
                    nc.sync.dma_start(
                        ain.ap()[b * src_rows:(b + 1) * src_rows, :],
                        src.ap()[:, b * W:(b + 1) * W])
                nc.gpsimd.collective_compute(
                    "AllToAll", ALU.bypass, replica_groups=RG,
                    ins=[ain.ap().opt()], outs=[dst.ap().opt()])

            # Persistent SBUF staging for the row-shard outputs that feed
            # collectives: lets us emit a few large DMAs instead of
            # thousands of 512B strided packets.
            aj_sb = const.tile([128, W // 128, HR], bf16)
            h1_sb = const.tile([128, WH // 128, HR], bf16)
            t2_sb = const.tile([128, W // 128, HR], bf16)

            def adjt_post(nc_, sbuf, md):
                # sbuf: f32 [128, 2, n_slice]; bf16 diag-fixed copy into
                # the persistent staging tile
                nsl = sbuf.shape[-1]
                n0 = md.n_tile_idx * md.n_tile
                sl = aj_sb[:, :, n0:n0 + nsl]
                nc_.vector.tensor_copy(sl, sbuf[:])
                diag_fix(sl, 0, md)

            def stage_half(src_sb, rows, dram_half, h):
                # src_sb half h -> contiguous [rows, HH] AG input
                nc.sync.dma_start(
                    dram_half.ap().rearrange("(s p) n -> p s n", p=128),
                    src_sb[:, :, h * HH:(h + 1) * HH])

            def stage_a2a(src_sb, rows, a2a_in):
                for b in range(NCORES):
                    nc.sync.dma_start(
                        a2a_in.ap()[b * rows:(b + 1) * rows, :].rearrange(
                            "(s p) j -> p s j", p=128),
                        src_sb[:, :, b * W:(b + 1) * W])

            # S1: Mc = U.T @ netouts_c     [LR, W]
            mmk(U_in.ap(), NOc.ap(), Mc.ap())

            # S2: adjT rows = |Mc.T @ aT|  [W, HR]  (f32 out, bf16 fixed copy)
            mmk(Mc.ap(), AT.ap(), ADJT_OUT.ap(), reducer=dve_abs,
                post=adjt_post, psum_bufs=4)
            stage_half(aj_sb, W, AJh[0], 0)
            ag(AJh[0], Bfh[0])          # fires while S2's 2nd half computes
            stage_a2a(aj_sb, W, BAin)
            nc.gpsimd.collective_compute(
                "AllToAll", ALU.bypass, replica_groups=RG,
                ins=[BAin.ap().opt()], outs=[Bcs.ap().opt()])
            stage_half(aj_sb, W, AJh[1], 1)
            ag(AJh[1], Bfh[1])

            # SBUF-resident copy of B: filled on first use (S3's kxn
            # producer DMAs each tile once), reused by S5 and S8.
            bf_sb = const.tile([128, HR // 128, HR], bf16)
            Bf_t = [b.ap().rearrange("(ko p) n -> p ko n", p=128)
                    for b in Bfh]

            def bf_fill_producer(h):
                def prod(nc_, md):
                    n0 = md.n_tile_idx * md.n_tile
                    ksl = bts(md.k_tile_idx, md.k_subtiles)
                    sl = bf_sb[:, ksl, h * HH + n0:h * HH + n0 + md.n_tile]
                    nc_.sync.dma_start(sl, Bf_t[h][:, ksl, n0:n0 + md.n_tile])
                    return sl
                return prod

            bf_hshape = ShapeInfo(pdims=((128, HR // 128),), fdims=(HH,))
            bf_cache = (bf_sb, HR, 0, HR)

            # S3: Z rows = |Bcs.T @ B|     [W, HR]  (bf16, diag fixed);
            # half h consumes AG1 half h, produces AG2 half h
            for h in range(2):
                def z_post(nc_, sbuf, md, h=h):
                    diag_fix(sbuf, h * HH, md)
                mmk(Bcs.ap(), None, Zch[h].ap(), reducer=dve_abs,
                    post=z_post, psum_bufs=4,
                    kxn_producer_shape=(bf_fill_producer(h), bf_hshape))
                ag(Zch[h], Zfh[h])

            # S4: T1 = Zf.T @ gc1c         [HR, WH]  (row half per Zf half)
            for h in range(2):
                mmk(Zfh[h].ap(), GC1c.ap(),
                    T1c.ap()[h * HH:(h + 1) * HH, :])

            # S5: h1T rows = relu(T1.T @ B) [WH, HR]
            def h1_post(nc_, sbuf, md):
                n0 = md.n_tile_idx * md.n_tile
                nc_.vector.tensor_copy(
                    h1_sb[:, :, n0:n0 + sbuf.shape[-1]], sbuf[:])

            mmk(T1c.ap(), None, H1c.ap(), reducer=dve_relu,
                kxn_cache_sb=bf_cache, post=h1_post, psum_bufs=4)

            # H1cs = h1T full [:, c*W:(c+1)*W] (A2A; no AllGather of h1T is
            # needed — its only consumer is this column slice)
            stage_a2a(h1_sb, WH, HAin)
            nc.gpsimd.collective_compute(
                "AllToAll", ALU.bypass, replica_groups=RG,
                ins=[HAin.ap().opt()], outs=[H1cs.ap().opt()])

            # S6: T2 rows = H1cs.T @ gc2   [W, HR]  (half per AG4 half)
            for h in range(2):
                def t2_post(nc_, sbuf, md, h=h):
                    n0 = h * HH + md.n_tile_idx * md.n_tile
                    nc_.vector.tensor_copy(
                        t2_sb[:, :, n0:n0 + sbuf.shape[-1]], sbuf[:])
                mmk(H1cs.ap(), GC2.ap()[:, h * HH:(h + 1) * HH],
                    T2ch[h].ap(), post=t2_post, psum_bufs=4)
                if h == 0:
                    ag(T2ch[0], T2fh[0])    # fires while S6's 2nd half runs
            stage_a2a(t2_sb, W, TAin)
            nc.gpsimd.collective_compute(
                "AllToAll", ALU.bypass, replica_groups=RG,
                ins=[TAin.ap().opt()], outs=[T2cs.ap().opt()])
            ag(T2ch[1], T2fh[1])

            # S7: X = 0.5*relu(Bcs.T @ T2f)   [W, HR] f32 (h2 rows, halved)
            for h in range(2):
                mmk(Bcs.ap(), T2fh[h].ap(),
                    Xc.ap()[:, h * HH:(h + 1) * HH],
                    reducer=dve_relu_half, psum_bufs=4)

            # S8: z rows = 0.5*relu(T2cs.T @ B) + X   [W, HR] f32
            for h in range(2):
                mmk(T2cs.ap(), None,
                    Z_OUT.ap()[:, h * HH:(h + 1) * HH],
                    reducer=dve_relu_half,
                    accum_ap=Xc.ap()[:, h * HH:(h + 1) * HH],
                    kxn_cache_sb=(bf_sb, HR, h * HH, HH), psum_bufs=4)

    nc.compile()
    return nc


def _get_nc():
    if "nc" not in _CACHE:
        _CACHE["nc"] = _build_nc()
    return _CACHE["nc"]


def _make_in_maps(U, net_outs, gsr_w, gc1_w, gc2_w):
    import ml_dtypes
    bf = ml_dtypes.bfloat16

    aT = np.ascontiguousarray((gsr_w[:, :LR] + gsr_w[:, LR:]).T).astype(bf)
    U_bf = U.astype(bf)
    gc2_bf = gc2_w.astype(bf)

    in_maps = []
    for c in range(NCORES):
        dmask = np.zeros((W, HR), np.float32)
        dmask[np.arange(W), c * W + np.arange(W)] = 1.0
        in_maps.append({
            "u": U_bf,
            "netouts_c": np.ascontiguousarray(
                net_outs[:, c * W:(c + 1) * W]).astype(bf),
            "at": aT,
            "gc1c": np.ascontiguousarray(
                gc1_w[:, c * WH:(c + 1) * WH]).astype(bf),
            "gc2": gc2_bf,
            "dmask": dmask.astype(bf),
        })
    return in_maps


def kernel(lr, gsr_w, start_w, start_b, down_w, down_b, pool_w, pool_b,
           bottom_w, bottom_b, end_w, end_b, up_w, up_b, gc1_w, gc2_w,
           lr_dim, hr_dim):
    global LAST_EXEC_NS
    from concourse.bass_utils import run_bass_kernel_spmd

    net_outs, start_outs, U, _A = _host_prefix(
        lr, start_w, start_b, down_w, down_b, pool_w, pool_b,
        bottom_w, bottom_b, up_w, up_b, end_w, end_b)

    nc = _get_nc()
    in_maps = _make_in_maps(U, net_outs, gsr_w, gc1_w, gc2_w)
    res = run_bass_kernel_spmd(nc, in_maps, list(range(NCORES)), trace=TRACE)
    LAST_EXEC_NS = res.exec_time_ns

    adjT = np.concatenate([res.results[c]["adjt"] for c in range(NCORES)], 0)
    z = np.concatenate([res.results[c]["zrows"] for c in range(NCORES)], 0)
    di = np.arange(HR)
    adj = np.ascontiguousarray(adjT.T)
    adj[di, di] = 1.0
    z[di, di] = 1.0
    return (z.astype(np.float32), net_outs.astype(np.float32),
            start_outs.astype(np.float32), adj.astype(np.float32))
